# revision 2
# baseline (speedup 1.0000x reference)
"""Multi-head attention (b=4, n=2048, dim=768, 12 heads) on 8 TRN2 NeuronCores.

Sharding: core c handles batch c//2 and head-group c%2 (6 of 12 heads).  Each
core computes its heads' contribution projected through its slice of Wo and
returns a partial [2048, 768] f32 output; the host sums core pairs and adds
the bias.  No on-device collectives needed.

Per-core kernel (TensorE-facing data in bf16, accumulation in f32):
  P1: KT/QT = W^T x^T feature-major with head pairs stacked 64+64 in the
      partition dim; V token-major [128 j, 384 f].
  P2: per (ib, pair, jc): scores computed TRANSPOSED ST[j,i] = K Q^T with the
      two heads of a pair issued as K=64 matmuls at row bases 0/64 (row-group
      concurrency); exp on ACT (scale 1/8) for 2/3 of tiles and on DVE via a
      Schraudolph bf16 bit-trick (tensor_scalar f32->u16 RNE) for 1/3;
      AV as col-paired M=64 matmuls into one [128,512] PSUM bank (head A in
      partitions 0:64, head B 64:128) accumulated over jc; softmax denominator
      from an all-ones [128,64] lhsT matmul pair into a second bank, also
      PSUM-accumulated and naturally replicated across 64 partitions.
  P3: normalize = DVE reciprocal(L) then OP*Linv -> bf16 feature-major otn
      (no transposes needed); output projection accumulates the three
      feature-chunks through Wo in PSUM; DVE copies to SBUF; DMA out.
"""
import os
import sys
import types
import numpy as np
import ml_dtypes

B, N, DIM = 4, 2048, 768
HEADS, DH = 12, 64
HPC = 6                # heads per core
FPC = HPC * DH         # 384 features per core
NCORES = 8
KC = DIM // 128        # 6 contraction chunks
FT = FPC // 128        # 3 feature tiles per core
NT = N // 128          # 16 key chunks of 128
IBS = 512              # i-block size
IB = N // IBS          # 4 i-blocks
NPAIR = HPC // 2       # 3 head pairs
BF16 = ml_dtypes.bfloat16

# Schraudolph bf16 exp: exp(s/8) ~= bitcast_u16(rne(s*A + B)); DVE convert
# is round-to-nearest (probed), c=-5.5 centers the error band (~+-3.3%).
EXP_A = float(0.125 * 128 / np.log(2.0))
EXP_B = float(16256 - 5.5)
DVE_EXP_MOD = 3        # units with u % MOD == MOD-1 do exp on DVE

_cache = {}
last_exec_time_ns = None


def _install_ntff_hook():
    try:
        import antenv.axon_hooks  # noqa: F401
        return
    except ImportError:
        pass
    from trn_agent_boot.trn_boot import _ntff_profile_via_ctypes
    hook = _ntff_profile_via_ctypes('/opt/axon/libaxon_pjrt.so')
    mod = types.ModuleType('antenv.axon_hooks')
    mod.get_axon_ntff_profile_hook = lambda: hook
    import antenv
    sys.modules['antenv.axon_hooks'] = mod
    antenv.axon_hooks = mod


def _build_nc():
    from contextlib import ExitStack
    from concourse import bacc
    import concourse.mybir as mybir
    from concourse.tile import TileContext

    dt = mybir.dt
    EXP = mybir.ActivationFunctionType.Exp
    ALU = mybir.AluOpType

    nc = bacc.Bacc("TRN2", target_bir_lowering=False, debug=False,
                   num_devices=NCORES)
    xT = nc.dram_tensor("xT", [DIM, N], dt.bfloat16, kind="ExternalInput").ap()
    wq = nc.dram_tensor("wq", [DIM, FPC], dt.bfloat16, kind="ExternalInput").ap()
    wk = nc.dram_tensor("wk", [DIM, FPC], dt.bfloat16, kind="ExternalInput").ap()
    wv = nc.dram_tensor("wv", [DIM, FPC], dt.bfloat16, kind="ExternalInput").ap()
    wo = nc.dram_tensor("wo", [FPC, DIM], dt.bfloat16, kind="ExternalInput").ap()
    out = nc.dram_tensor("out", [N, DIM], dt.float32, kind="ExternalOutput").ap()

    with TileContext(nc) as tc, ExitStack() as ctx:
        const = ctx.enter_context(tc.tile_pool(name="const", bufs=1))
        ones = const.tile([128, 64], dt.bfloat16, tag="ones", name="ones")
        nc.vector.memset(ones[:], 1.0)

        inp = ctx.enter_context(tc.tile_pool(name="inp", bufs=1))
        xts2 = [[inp.tile([128, N // 2], dt.bfloat16, tag=f"xt{k}_{hf}",
                          name=f"xt{k}_{hf}") for hf in range(2)]
                for k in range(KC)]
        wqs = [inp.tile([128, FPC], dt.bfloat16, tag=f"wq{k}", name=f"wq{k}")
               for k in range(KC)]
        wks = [inp.tile([128, FPC], dt.bfloat16, tag=f"wk{k}", name=f"wk{k}")
               for k in range(KC)]
        wvs = [inp.tile([128, FPC], dt.bfloat16, tag=f"wv{k}", name=f"wv{k}")
               for k in range(KC)]
        wos = [inp.tile([128, DIM], dt.bfloat16, tag=f"wo{f}", name=f"wo{f}")
               for f in range(FT)]
        for k in range(KC):
            nc.sync.dma_start(out=xts2[k][0][:],
                              in_=xT[k * 128:(k + 1) * 128, 0:N // 2])
            nc.scalar.dma_start(out=wvs[k][:], in_=wv[k * 128:(k + 1) * 128, :])
        for k in range(KC):
            nc.sync.dma_start(out=xts2[k][1][:],
                              in_=xT[k * 128:(k + 1) * 128, N // 2:N])
        for k in range(KC):
            nc.sync.dma_start(out=wks[k][:], in_=wk[k * 128:(k + 1) * 128, :])
            nc.sync.dma_start(out=wqs[k][:], in_=wq[k * 128:(k + 1) * 128, :])
        for f in range(FT):
            nc.scalar.dma_start(out=wos[f][:], in_=wo[f * 128:(f + 1) * 128, :])

        kqv = ctx.enter_context(tc.tile_pool(name="kqv", bufs=1))
        KT = [kqv.tile([128, N], dt.bfloat16, tag=f"kt{f}", name=f"kt{f}")
              for f in range(FT)]
        QT = [kqv.tile([128, N], dt.bfloat16, tag=f"qt{f}", name=f"qt{f}")
              for f in range(FT)]
        VP = [kqv.tile([128, FPC], dt.bfloat16, tag=f"vp{t}", name=f"vp{t}")
              for t in range(NT)]

        # ---- P1: projections ----
        with tc.tile_pool(name="p1ps", bufs=3, space="PSUM") as p1:
            for t in range(NT):
                ps = p1.tile([128, FPC], dt.float32, tag="p1", name=f"vps{t}")
                for k in range(KC):
                    nc.tensor.matmul(
                        ps[:],
                        lhsT=xts2[k][t // 8][:, (t % 8) * 128:(t % 8 + 1) * 128],
                        rhs=wvs[k][:], start=(k == 0), stop=(k == KC - 1))
                nc.vector.tensor_copy(VP[t][:], ps[:])
            for W, DST in ((wks, KT), (wqs, QT)):
                for f in range(FT):
                    for q in range(N // 512):
                        ps = p1.tile([128, 512], dt.float32, tag="p1",
                                     name=f"kqps{f}_{q}")
                        for k in range(KC):
                            nc.tensor.matmul(
                                ps[:], lhsT=W[k][:, f * 128:(f + 1) * 128],
                                rhs=xts2[k][q // 2][:, (q % 2) * 512:
                                                    (q % 2 + 1) * 512],
                                start=(k == 0), stop=(k == KC - 1))
                        nc.vector.tensor_copy(DST[f][:, q * 512:(q + 1) * 512],
                                              ps[:])

        # ---- P2 + P3 ----
        with tc.tile_pool(name="stp", bufs=2, space="PSUM") as stp, \
                tc.tile_pool(name="avp", bufs=2, space="PSUM") as avp, \
                tc.tile_pool(name="lpp", bufs=2, space="PSUM") as lpp, \
                tc.tile_pool(name="expp", bufs=4) as expp, \
                tc.tile_pool(name="linvp", bufs=2) as linvp, \
                tc.tile_pool(name="otnp", bufs=4) as otnp, \
                tc.tile_pool(name="obp", bufs=3) as obp:
            sts = {}
            exs = {}
            avt = {}
            lpt = {}
            otns = {}

            def emit_scores(ib, p, jc):
                st = stp.tile([128, 1024], dt.float32, tag="st",
                              name=f"st{ib}_{p}_{jc}")
                for hh, r0 in ((0, 0), (1, 64)):
                    nc.tensor.matmul(
                        st[:, hh * 512:(hh + 1) * 512],
                        lhsT=KT[p][r0:r0 + 64, jc * 128:(jc + 1) * 128],
                        rhs=QT[p][r0:r0 + 64, ib * IBS:(ib + 1) * IBS],
                        start=True, stop=True)
                sts[(p, jc)] = st

            def emit_exp(ib, p, jc, u):
                st = sts.pop((p, jc))
                ex = expp.tile([128, 1024], dt.bfloat16, tag="ex",
                               name=f"ex{ib}_{p}_{jc}")
                if u % DVE_EXP_MOD == DVE_EXP_MOD - 1:
                    nc.vector.tensor_scalar(ex[:].bitcast(dt.uint16), st[:],
                                            EXP_A, EXP_B, ALU.mult, ALU.add)
                else:
                    nc.scalar.activation(ex[:], st[:], EXP, scale=0.125)
                exs[(p, jc)] = ex

            def emit_av(ib, p, jc):
                ex = exs.pop((p, jc))
                if jc == 0:
                    avt[p] = avp.tile([128, 512], dt.float32, tag="av",
                                      name=f"av{ib}_{p}")
                    lpt[p] = lpp.tile([128, 512], dt.float32, tag="lp",
                                      name=f"lp{ib}_{p}")
                av, lp = avt[p], lpt[p]
                first, last = (jc == 0), (jc == NT - 1)
                for hh in range(2):
                    h = 2 * p + hh
                    nc.tensor.matmul(
                        av[hh * 64:(hh + 1) * 64, :],
                        lhsT=VP[jc][:, h * 64:(h + 1) * 64],
                        rhs=ex[:, hh * 512:(hh + 1) * 512],
                        start=first, stop=last)
                    nc.tensor.matmul(
                        lp[hh * 64:(hh + 1) * 64, :], lhsT=ones[:],
                        rhs=ex[:, hh * 512:(hh + 1) * 512],
                        start=first, stop=last)
                if last:
                    av, lp = avt.pop(p), lpt.pop(p)
                    linv = linvp.tile([128, 512], dt.float32, tag="linv",
                                      name=f"linv{ib}_{p}")
                    nc.vector.reciprocal(linv[:], lp[:])
                    otn = otnp.tile([128, 512], dt.bfloat16, tag="otn",
                                    name=f"otn{ib}_{p}")
                    nc.vector.tensor_tensor(otn[:], av[:], linv[:], ALU.mult)
                    otns[(ib, p)] = otn

            def emit_proj(ib):
                for isub in range(IBS // 128):
                    ob = obp.tile([128, DIM], dt.float32, tag="ob",
                                  name=f"ob{ib}_{isub}")
                    for half in range(2):
                        pp = avp.tile([128, 512], dt.float32, tag="av",
                                      name=f"pp{ib}_{isub}_{half}")
                        for p in range(NPAIR):
                            nc.tensor.matmul(
                                pp[:, 0:384],
                                lhsT=otns[(ib, p)][:, isub * 128:
                                                   (isub + 1) * 128],
                                rhs=wos[p][:, half * 384:(half + 1) * 384],
                                start=(p == 0), stop=(p == NPAIR - 1))
                        nc.vector.tensor_copy(ob[:, half * 384:(half + 1) * 384],
                                              pp[:, 0:384])
                    nc.sync.dma_start(
                        out=out[(ib * 4 + isub) * 128:(ib * 4 + isub + 1) * 128,
                                :],
                        in_=ob[:])
                for p in range(NPAIR):
                    otns.pop((ib, p))

            u = 0
            for ib in range(IB):
                seq = [(p, jc) for p in range(NPAIR) for jc in range(NT)]
                for i in range(len(seq) + 1):
                    if i < len(seq):
                        emit_scores(ib, *seq[i])
                        emit_exp(ib, *seq[i], u)
                        u += 1
                    if i >= 1:
                        emit_av(ib, *seq[i - 1])
                emit_proj(ib)

    nc.finalize()
    return nc


def _get_nc():
    if "nc" not in _cache:
        _cache["nc"] = _build_nc()
    return _cache["nc"]


def kernel(x, Wq, Wk, Wv, Wo, bo):
    global last_exec_time_ns
    x = np.asarray(x, dtype=np.float32)
    Wq = np.asarray(Wq, dtype=np.float32)
    Wk = np.asarray(Wk, dtype=np.float32)
    Wv = np.asarray(Wv, dtype=np.float32)
    Wo = np.asarray(Wo, dtype=np.float32)
    bo = np.asarray(bo, dtype=np.float32)

    trace = bool(os.environ.get("BASS_KERNEL_TRACE"))
    if trace:
        _install_ntff_hook()
        import concourse.bass_utils as bass_utils
        bass_utils.upload_artifacts = lambda tmpdir: tmpdir

    nc = _get_nc()
    in_maps = []
    for c in range(NCORES):
        bi, hg = divmod(c, 2)
        s = slice(hg * FPC, (hg + 1) * FPC)
        in_maps.append({
            "xT": np.ascontiguousarray(x[bi].T).astype(BF16),
            "wq": np.ascontiguousarray(Wq[:, s]).astype(BF16),
            "wk": np.ascontiguousarray(Wk[:, s]).astype(BF16),
            "wv": np.ascontiguousarray(Wv[:, s]).astype(BF16),
            "wo": np.ascontiguousarray(Wo[s, :]).astype(BF16),
        })

    from concourse.bass_utils import run_bass_kernel_spmd
    res = run_bass_kernel_spmd(nc, in_maps, list(range(NCORES)), trace=trace)
    last_exec_time_ns = res.exec_time_ns

    parts = [res.results[c]["out"] for c in range(NCORES)]
    full = np.empty((B, N, DIM), np.float32)
    for bi in range(B):
        full[bi] = parts[2 * bi] + parts[2 * bi + 1] + bo[None, :]
    return full
